# revision 10
# baseline (speedup 1.0000x reference)
"""Causal self-attention Trainium2 kernel.

Problem: B=2, T=2048, C=768, 12 heads of dim 64, fp32.
  qkv = x @ W_attn.T ; per-head causal softmax(Q K^T / 8) @ V ; y = attn @ W_proj.T

Sharding over 8 cores: core = b * 4 + g where b = batch (2), g = head-group
(4 groups x 3 heads).  Each core computes QKV for its 3 heads, causal
attention, and a partial projection y_partial[b] = attn[:, S_g] @ W_proj[:, S_g].T.
Host sums the 4 partials per batch (the gather/unshard step).

Layout (no on-device transposes needed anywhere):
  - host passes x[b].T (xT [768, 2048]), W_q/k/v rows^T ([768, 192]) and
    W_proj cols^T (wpT [192, 768]).
  - Q^T, K^T are computed d-major [64, T] (packed into one [128, T] tile);
    V t-major with an appended ones-column so the P@V matmul also emits the
    softmax denominator as its output row 64.
  - scores are computed transposed, ST[k, q] = K Q^T; exp runs on ScalarE
    straight out of PSUM (no max-subtraction: |scores/8| < ~2 here, safe in
    fp32; masked lanes get -1e30 and underflow to exact 0).
  - causal masking happens ON TensorE: the diagonal-region score group
    accumulates identity.T @ mneg_slice (host-provided -1e30 band matrix)
    into the same PSUM bank.
  - softmax normalization: reciprocal of the denominator row, broadcast
    across partitions with a K=1 outer-product matmul, one multiply.

This walrus rejects any engine instruction carrying >= 2 semaphore waits, and
Tile emits one wait per distinct required semaphore (no cross-proc dedup, only
per-engine observed-clock dedup).  The kernel is therefore arranged so every
engine instruction needs at most ONE newly-observed semaphore:
  - DMA issue order pairs co-consumed tensors on the same DMAHW lane,
  - PSUM evacuations of q/k/v are pinned to ScalarE, the normalize chain and
    output evacuations to VectorE,
  - output DMAs go through SWDGE (gpsimd) lanes, one per chunk, never reused,
  - at phase boundaries a 1x1 "carrier" matmul (value later overwritten by the
    group's start=True clear) observes one semaphore ahead of the real work.
"""

from contextlib import ExitStack

import numpy as np

import concourse.bass as bass
import concourse.mybir as mybir
from concourse.tile import TileContext
from concourse.bass_utils import run_bass_kernel_spmd

B, T, C = 2, 2048, 768
NH = 12
HEAD = 64
HPC = 3              # heads per core
CP = HPC * HEAD      # 192 channels per core
SCALE = 1.0 / 8.0    # 1/sqrt(64)
NEG = -1.0e30

P = 128
TT = T // P          # 16 t-tiles
CK = C // P          # 6 contraction chunks over C
QC = 512             # q-chunk (one PSUM bank of fp32)
NQC = T // QC        # 4
MNW = 896            # mneg width: cols 384 - 128*m + q for m in 0..3
NYC = 8              # output chunks (one SWDGE lane each)
TPY = TT // NYC      # t-tiles per output chunk (2)
F32 = mybir.dt.float32

_CACHED = {}


def _split_excess_waits(nc):
    """This walrus accepts at most 1 semaphore wait per instruction (2 on
    EventSemaphore).  Move excess waits onto same-engine EventSemaphore
    instructions inserted immediately before the overloaded instruction —
    sequencer FIFO order makes that semantically identical."""
    n = 0
    for f in nc.m.functions:
        for bb in f.blocks:
            out = []
            for inst in bb.instructions:
                cap = 2 if type(inst).__name__ == "InstEventSemaphore" else 1
                si = inst.sync_info
                if si is not None and len(si.on_wait) > cap:
                    waits = list(si.on_wait)
                    extra, keep = waits[:-cap], waits[-cap:]
                    while extra:
                        chunk, extra = extra[:2], extra[2:]
                        n += 1
                        ev = mybir.InstEventSemaphore(
                            name=f"WSPLIT-{n}", engine=inst.engine
                        )
                        ev.sync_info = mybir.SyncInfo(on_wait=chunk, on_update=[])
                        out.append(ev)
                    inst.sync_info = mybir.SyncInfo(
                        on_wait=keep, on_update=list(si.on_update)
                    )
                out.append(inst)
            bb.instructions = out
    return n


def _build():
    nc = bass.Bass()

    xT = nc.dram_tensor("xT", [C, T], F32, kind="ExternalInput")
    wqT = nc.dram_tensor("wqT", [C, CP], F32, kind="ExternalInput")
    wkT = nc.dram_tensor("wkT", [C, CP], F32, kind="ExternalInput")
    wvT = nc.dram_tensor("wvT", [C, CP], F32, kind="ExternalInput")
    wpT = nc.dram_tensor("wpT", [CP, C], F32, kind="ExternalInput")
    ident = nc.dram_tensor("ident", [P, P], F32, kind="ExternalInput")
    mneg = nc.dram_tensor("mneg", [P, MNW], F32, kind="ExternalInput")
    y = nc.dram_tensor("y", [T, C], F32, kind="ExternalOutput")

    Exp = mybir.ActivationFunctionType.Exp
    Copy = mybir.ActivationFunctionType.Copy

    with TileContext(nc) as tc, ExitStack() as stk:

        def carrier(psum_ap, dep_ap, start=True):
            """1x1 matmul observing dep_ap's semaphore; lands in a PSUM cell
            that the next start=True matmul of the real group will clear."""
            nc.tensor.matmul(
                psum_ap,
                lhsT=dep_ap,
                rhs=dep_ap,
                start=start,
                stop=False,
                skip_group_check=True,
            )

        wpool = stk.enter_context(tc.tile_pool(name="weights", bufs=1))
        xpool = stk.enter_context(tc.tile_pool(name="xpool", bufs=1))
        vpool = stk.enter_context(tc.tile_pool(name="vpool", bufs=1))
        qkpool = stk.enter_context(tc.tile_pool(name="qkpool", bufs=2))
        otpool = stk.enter_context(tc.tile_pool(name="otpool", bufs=1))
        ptpool = stk.enter_context(tc.tile_pool(name="ptpool", bufs=2))
        misc = stk.enter_context(tc.tile_pool(name="misc", bufs=1))
        ypool = stk.enter_context(tc.tile_pool(name="ypool", bufs=4))
        ps_st = stk.enter_context(tc.tile_pool(name="ps_st", bufs=2, space="PSUM"))
        ps_ot = stk.enter_context(tc.tile_pool(name="ps_ot", bufs=1, space="PSUM"))
        ps_bc = stk.enter_context(tc.tile_pool(name="ps_bc", bufs=1, space="PSUM"))
        ps_sm = stk.enter_context(tc.tile_pool(name="ps_sm", bufs=2, space="PSUM"))

        # ---- tiles ----
        wq_sb = wpool.tile([P, CK, CP], F32)
        wk_sb = wpool.tile([P, CK, CP], F32)
        wv_sb = wpool.tile([P, CK, CP], F32)
        wp_sb = wpool.tile([HEAD, HPC, C], F32)
        id_sb = wpool.tile([P, P], F32)
        mn_sb = wpool.tile([P, MNW], F32)
        ones_sb = wpool.tile([1, HEAD], F32)
        x_sb = [
            xpool.tile([P, T], F32, name=f"x_ck{ck}", tag=f"x_ck{ck}")
            for ck in range(CK)
        ]

        # ---- DMA issue order == DMAHW lane (idx % 8):
        # wv:L0 x1:L1 x2:L2 ident:L3 wp:L4 x3:L5 x4:L6 x5:L7
        # x0:L0(pairs wv) wq:L1 wk:L2 mneg:L3(pairs ident)
        nc.sync.dma_start(wv_sb, wvT[:, :].rearrange("(ck p) o -> p ck o", p=P))
        nc.sync.dma_start(x_sb[1], xT[1 * P : 2 * P, :])
        nc.sync.dma_start(x_sb[2], xT[2 * P : 3 * P, :])
        nc.sync.dma_start(id_sb, ident[:, :])
        nc.sync.dma_start(wp_sb, wpT[:, :].rearrange("(h d) c -> d h c", d=HEAD))
        nc.sync.dma_start(x_sb[3], xT[3 * P : 4 * P, :])
        nc.sync.dma_start(x_sb[4], xT[4 * P : 5 * P, :])
        nc.sync.dma_start(x_sb[5], xT[5 * P : 6 * P, :])
        nc.sync.dma_start(x_sb[0], xT[0 * P : 1 * P, :])
        nc.sync.dma_start(wq_sb, wqT[:, :].rearrange("(ck p) o -> p ck o", p=P))
        nc.sync.dma_start(wk_sb, wkT[:, :].rearrange("(ck p) o -> p ck o", p=P))
        nc.sync.dma_start(mn_sb, mneg[:, :])

        nc.vector.memset(ones_sb, 1.0)

        # ---- V for all heads: t-major [t, (tile, h, d|1)] ----
        v_sb = vpool.tile([P, TT, HPC, HEAD + 1], F32)
        # ones column via ScalarE so v_sb is single-producer-engine:
        # out = Copy(0 * garbage + 1) = 1
        nc.scalar.activation(
            v_sb[:, :, :, HEAD : HEAD + 1],
            v_sb[:, :, :, HEAD : HEAD + 1],
            Copy,
            bias=1.0,
            scale=0.0,
        )
        for i in range(TT):
            pv = ps_sm.tile([P, QC], F32, tag="ps_sm", name="pv")
            for ck in range(CK):
                nc.tensor.matmul(
                    pv[:, :CP],
                    lhsT=x_sb[ck][:, i * P : (i + 1) * P],
                    rhs=wv_sb[:, ck, :],
                    start=(ck == 0),
                    stop=(ck == CK - 1),
                )
            nc.scalar.copy(
                out=v_sb[:, i, :, 0:HEAD],
                in_=pv[:, :CP].rearrange("p (h d) -> p h d", d=HEAD),
            )

        ot_sb = otpool.tile([HEAD, HPC, T], F32)

        for h in range(HPC):
            # ---- Q^T, K^T for head h: d-major [64, T] ----
            qt = qkpool.tile([HEAD, T], F32, tag="qt")
            kt = qkpool.tile([HEAD, T], F32, tag="kt")
            for n in range(NQC):
                pq = ps_sm.tile([P, QC], F32, tag="ps_sm", name="pq")
                if h == 0 and n == 0:
                    carrier(pq[0:1, 0:1], wq_sb[0:1, 0, 0:1])
                for ck in range(CK):
                    nc.tensor.matmul(
                        pq[:HEAD, :],
                        lhsT=wq_sb[:, ck, h * HEAD : (h + 1) * HEAD],
                        rhs=x_sb[ck][:, n * QC : (n + 1) * QC],
                        start=(ck == 0),
                        stop=(ck == CK - 1),
                    )
                nc.scalar.copy(out=qt[:, n * QC : (n + 1) * QC], in_=pq[:HEAD, :])
                pk = ps_sm.tile([P, QC], F32, tag="ps_sm", name="pk")
                if h == 0 and n == 0:
                    carrier(pk[0:1, 0:1], wk_sb[0:1, 0, 0:1])
                for ck in range(CK):
                    nc.tensor.matmul(
                        pk[:HEAD, :],
                        lhsT=wk_sb[:, ck, h * HEAD : (h + 1) * HEAD],
                        rhs=x_sb[ck][:, n * QC : (n + 1) * QC],
                        start=(ck == 0),
                        stop=(ck == CK - 1),
                    )
                nc.scalar.copy(out=kt[:, n * QC : (n + 1) * QC], in_=pk[:HEAD, :])

            # ---- attention for head h, per q-chunk ----
            for j in range(NQC):
                nkt = 4 * (j + 1)  # causal: k-tiles 0 .. 4j+3
                ot_ps = ps_ot.tile([HEAD + 1, QC], F32, tag="ot")
                for g in range(0, nkt, 2):
                    gsz = min(2, nkt - g)
                    st = ps_st.tile([P, 2, QC], F32, tag="st")
                    for u in range(gsz):
                        i = g + u
                        m = i - 4 * j
                        nc.tensor.matmul(
                            st[:, u, :],
                            lhsT=kt[:, i * P : (i + 1) * P],
                            rhs=qt[:, j * QC : (j + 1) * QC],
                            start=True,
                            stop=(m < 0),
                        )
                        if m >= 0:
                            # accumulate causal -1e30 mask via identity
                            nc.tensor.matmul(
                                st[:, u, :],
                                lhsT=id_sb,
                                rhs=mn_sb[:, 384 - 128 * m : 896 - 128 * m],
                                start=False,
                                stop=True,
                            )
                    pt = ptpool.tile([P, 2, QC], F32, tag="pt")
                    nc.scalar.activation(
                        pt[:, :gsz, :], st[:, :gsz, :], Exp, scale=SCALE
                    )
                    for u in range(gsz):
                        i = g + u
                        nc.tensor.matmul(
                            ot_ps,
                            lhsT=v_sb[:, i, h, :],
                            rhs=pt[:, u, :],
                            start=(i == 0),
                            stop=(i == nkt - 1),
                        )
                # normalize: row 64 of ot_ps is the softmax denominator.
                # copy BEFORE reciprocal: the bc matmul's DVE wait (>= rec)
                # then also covers the copy, so the ot_ps slot release is
                # already observed by PE when the next q-chunk starts.
                ot_tmp = misc.tile([HEAD, QC], F32, tag="ot_tmp")
                nc.vector.tensor_copy(out=ot_tmp, in_=ot_ps[0:HEAD, :])
                rec = misc.tile([1, QC], F32, tag="rec")
                nc.vector.reciprocal(rec, ot_ps[HEAD : HEAD + 1, :])
                bc = ps_bc.tile([HEAD, QC], F32, tag="bc")
                nc.tensor.matmul(bc, lhsT=ones_sb, rhs=rec, start=True, stop=True)
                nc.vector.tensor_mul(
                    ot_sb[:, h, j * QC : (j + 1) * QC], ot_tmp, bc
                )

        # ---- output projection ----
        for i in range(TT):
            isl = slice(i * P, (i + 1) * P)
            pa = ps_sm.tile([P, QC], F32, tag="ps_sm", name="pa")
            if i == 0:
                carrier(pa[0:1, 0:1], id_sb[0:1, 0:1])          # psum slot (ACT)
                carrier(pa[0:1, 1:2], wp_sb[0:1, 0, 0:1], False)  # wp DMA lane
                carrier(pa[0:1, 2:3], ot_sb[0:1, 0, 0:1], False)  # ot (DVE)
            for h in range(HPC):
                nc.tensor.matmul(
                    pa,
                    lhsT=ot_sb[:, h, isl],
                    rhs=wp_sb[:, h, 0:QC],
                    start=(h == 0),
                    stop=(h == HPC - 1),
                )
            y_sb = ypool.tile([P, C], F32, tag="ysb")
            nc.vector.tensor_copy(out=y_sb[:, 0:QC], in_=pa)
            pb = ps_sm.tile([P, QC], F32, tag="ps_sm", name="pb")
            for h in range(HPC):
                nc.tensor.matmul(
                    pb[:, : C - QC],
                    lhsT=ot_sb[:, h, isl],
                    rhs=wp_sb[:, h, QC:C],
                    start=(h == 0),
                    stop=(h == HPC - 1),
                )
            nc.vector.tensor_copy(out=y_sb[:, QC:C], in_=pb[:, : C - QC])
            nc.gpsimd.dma_start(y[isl, :], y_sb)

    _split_excess_waits(nc)
    return nc


def _in_maps(x, W_attn, W_proj):
    ident = np.eye(P, dtype=np.float32)
    mneg = np.where(
        np.arange(P)[:, None] > (np.arange(MNW)[None, :] - 384), NEG, 0.0
    ).astype(np.float32)
    maps = []
    for core in range(8):
        b, g = divmod(core, 4)
        s = slice(g * CP, (g + 1) * CP)
        maps.append(
            dict(
                xT=np.ascontiguousarray(x[b].T),
                wqT=np.ascontiguousarray(W_attn[0 * C :][s].T),
                wkT=np.ascontiguousarray(W_attn[1 * C :][s].T),
                wvT=np.ascontiguousarray(W_attn[2 * C :][s].T),
                wpT=np.ascontiguousarray(W_proj[:, s].T),
                ident=ident,
                mneg=mneg,
            )
        )
    return maps


def run(x, W_attn, W_proj, trace=False):
    if "nc" not in _CACHED:
        _CACHED["nc"] = _build()
    nc = _CACHED["nc"]
    res = run_bass_kernel_spmd(nc, _in_maps(x, W_attn, W_proj), list(range(8)), trace=trace)
    y = np.empty((B, T, C), dtype=np.float32)
    for b in range(B):
        y[b] = res.results[4 * b]["y"]
        for g in range(1, 4):
            y[b] += res.results[4 * b + g]["y"]
    return y, res


def kernel(x, W_attn, W_proj):
    x = np.asarray(x, dtype=np.float32)
    W_attn = np.asarray(W_attn, dtype=np.float32)
    W_proj = np.asarray(W_proj, dtype=np.float32)
    y, _ = run(x, W_attn, W_proj, trace=False)
    return y
